# revision 1
# baseline (speedup 1.0000x reference)
"""Trainium2 Bass kernel for nn_CXNGeneralLayer (GNN message passing).

z = relu(Gi2j @ (xi W_i + b_i) + Adj2j @ (xj1 W_j1 + b_j1)
         + coAdj2j @ (xj1 W_j2 + b_j2) + Gk2j @ (xk W_k + b_k))

Sharding (per the 1D row-parallel hint): output rows (n_j) are split
across 8 NeuronCores; each core streams its [1024, 8192] shard of all
four operator matrices, which dominate the traffic (128 MB/core). The
shards are pre-transposed on the host to [8192(t), 1024(j)] so the
contraction dim sits on SBUF partitions. The small activations
h_m = x_m W_m + b_m (1 MB each) are replicated/broadcast to every core
in stationary-operand layout, so z^T = sum_m h_m^T @ G_m^T accumulates
directly in PSUM with N=512 moving tiles. float32r operands keep the
DMA stream bit-identical fp32 while the PE runs single-pass matmuls.
"""

import sys

import numpy as np

if "/opt/trn_rl_repo" not in sys.path:
    sys.path.insert(0, "/opt/trn_rl_repo")

N = 8192  # n_i = n_j = n_k
C = 32  # c_in = c_out
N_CORES = 8
JS = N // N_CORES  # 1024 output rows per core
KP = 128  # contraction partition tile
KCH = N // KP  # 64 t-chunks
NJH = 2  # j-halves of 512 (fp32 moving-operand max)

_compiled = None


def _build_program():
    import concourse.mybir as mybir
    import concourse.tile as tile
    from concourse import bacc

    f32 = mybir.dt.float32
    f32r = mybir.dt.float32r  # same bits as f32; 1-pass PE matmul (vs LOW_HIGH)
    nc = bacc.Bacc("TRN2", target_bir_lowering=False)

    gts = [
        nc.dram_tensor(f"gt{m}", [N, JS], f32r, kind="ExternalInput") for m in range(4)
    ]
    # h_m in stationary layout: hs[m][p, 32k+c] = h_m[128k+p, c]
    hs = [
        nc.dram_tensor(f"h{m}", [KP, KCH * C], f32r, kind="ExternalInput")
        for m in range(4)
    ]
    out_t = nc.dram_tensor("outT", [C, JS], f32, kind="ExternalOutput")

    with tile.TileContext(nc) as tc:
        with (
            tc.tile_pool(name="cpool", bufs=1) as cpool,
            tc.tile_pool(name="gpool", bufs=32) as gpool,
            tc.tile_pool(name="zpsum", bufs=2, space="PSUM") as zpsum,
        ):
            h_sb = []
            for m in range(4):
                h = cpool.tile([KP, KCH * C], f32r, tag=f"h{m}", name=f"h{m}")
                # SWDGE queue: keeps the HWDGE rings free for the G stream
                nc.gpsimd.dma_start(h[:], hs[m][:])
                h_sb.append(h)

            # z^T[c, j] += sum_t h_m[t, c] * G_m^T[t, j], streaming G^T in
            # 512 KB tiles; one PSUM accumulation group per 512-wide j-half
            # spanning all 4 matrices x 64 chunks.
            zp = [
                zpsum.tile([C, 512], f32, tag=f"zp{jh}", name=f"zp{jh}")
                for jh in range(NJH)
            ]
            for m in range(4):
                for k in range(KCH):
                    gt = gpool.tile([KP, JS], f32r, tag="gt")
                    dma_eng = nc.sync if k % 2 == 0 else nc.scalar
                    dma_eng.dma_start(gt[:], gts[m][KP * k : KP * (k + 1), :])
                    first = m == 0 and k == 0
                    last = m == 3 and k == KCH - 1
                    for jh in range(NJH):
                        nc.tensor.matmul(
                            zp[jh][:],
                            h_sb[m][:, C * k : C * (k + 1)],
                            gt[:, 512 * jh : 512 * (jh + 1)],
                            start=first,
                            stop=last,
                        )

            # relu and store z^T shard; per-half so the first store
            # overlaps the other half's final matmul
            zsb = cpool.tile([C, JS], f32, tag="zsb")
            for jh in range(NJH):
                nc.scalar.activation(
                    zsb[:, 512 * jh : 512 * (jh + 1)],
                    zp[jh][:],
                    mybir.ActivationFunctionType.Relu,
                )
                nc.sync.dma_start(
                    out_t[:, 512 * jh : 512 * (jh + 1)],
                    zsb[:, 512 * jh : 512 * (jh + 1)],
                )

    nc.compile()
    return nc


def _get_program():
    global _compiled
    if _compiled is None:
        _compiled = _build_program()
    return _compiled


def _prep_inputs(inputs):
    """Host-side sharding: returns per-core input maps."""
    f32 = np.float32
    branches = [
        ("Gi2j", "xi", "W_i", "b_i"),
        ("Adj2j", "xj1", "W_j1", "b_j1"),
        ("coAdj2j", "xj1", "W_j2", "b_j2"),
        ("Gk2j", "xk", "W_k", "b_k"),
    ]
    shared = {}
    for m, (_, xn, wn, bn) in enumerate(branches):
        x = np.asarray(inputs[xn], dtype=f32)
        w = np.asarray(inputs[wn], dtype=f32)
        b = np.asarray(inputs[bn], dtype=f32)
        h = x @ w + b  # [N, C] replicated activation, broadcast to all cores
        shared[f"h{m}"] = np.ascontiguousarray(
            h.reshape(KCH, KP, C).transpose(1, 0, 2).reshape(KP, KCH * C)
        )

    in_maps = []
    for s in range(N_CORES):
        im = dict(shared)
        for m, (gn, _, _, _) in enumerate(branches):
            g = np.asarray(inputs[gn])
            im[f"gt{m}"] = np.ascontiguousarray(g[s * JS : (s + 1) * JS, :].T, dtype=f32)
        in_maps.append(im)
    return in_maps


def _run(inputs, trace=False):
    from concourse.bass_utils import run_bass_kernel_spmd

    nc = _get_program()
    in_maps = _prep_inputs(inputs)
    try:
        res = run_bass_kernel_spmd(nc, in_maps, list(range(N_CORES)), trace=trace)
    except Exception:
        # transient device errors (e.g. NRT_EXEC_UNIT_UNRECOVERABLE) clear
        # on re-dispatch; retry once before giving up
        res = run_bass_kernel_spmd(nc, in_maps, list(range(N_CORES)), trace=trace)
    out = np.concatenate(
        [res.results[s]["outT"] for s in range(N_CORES)], axis=1
    ).T
    return np.ascontiguousarray(out, dtype=np.float32), res


def kernel(**inputs):
    out, _ = _run(inputs, trace=False)
    return out



# revision 6
# speedup vs baseline: 1.9322x; 1.9322x over previous
"""Trainium2 Bass kernel for nn_CXNGeneralLayer (GNN message passing).

z = relu(Gi2j @ (xi W_i + b_i) + Adj2j @ (xj1 W_j1 + b_j1)
         + coAdj2j @ (xj1 W_j2 + b_j2) + Gk2j @ (xk W_k + b_k))

Sharding (per the 1D row-parallel hint): output rows (n_j) are split
across 8 NeuronCores; each core streams its [1024, 8192] shard of all
four operator matrices, which dominate the traffic. Shards are
pre-transposed on the host (contraction dim on SBUF partitions) and
cast to bf16, halving HBM traffic to 64 MB/core; bf16 rounding
contributes ~2.3e-3 relative output error (measured offline), 8x
inside the 2e-2 gate. DMA tiles span 4 contraction chunks (1 MiB,
8 KiB contiguous per partition line) for near-line-rate streaming.

The PE array runs in 128x32 column-tiled mode: the four matrices'
matmuls execute CONCURRENTLY in the four 32-column strips
(tile_position=(0, 32m)), each strip accumulating its branch partial
h_m^T @ G_m^T into its own partition range of a shared PSUM bank.
This lifts array utilization 4x so the PE streams ~4x fewer wall-ns
than the naive layout and DMA becomes the sole bottleneck. The four
partials are then cross-partition summed by one selector matmul
(S[32m+c, c'] = delta(c, c')), relu'd, and stored.
"""

import sys

import numpy as np

if "/opt/trn_rl_repo" not in sys.path:
    sys.path.insert(0, "/opt/trn_rl_repo")

N = 8192  # n_i = n_j = n_k
C = 32  # c_in = c_out
N_CORES = 8
JS = N // N_CORES  # 1024 output rows per core
KP = 128  # contraction partition tile
KCH = N // KP  # 64 t-chunks
BLK = 4  # t-chunks per DMA tile (1 MiB bf16)
NBLK = KCH // BLK  # 16 block DMAs per matrix
NJH = 2  # j-halves of 512 (PSUM-bank moving-operand max)

_compiled = None


def _build_program():
    import concourse.mybir as mybir
    import concourse.tile as tile
    from concourse import bacc

    f32 = mybir.dt.float32
    f32r = mybir.dt.float32r
    bf16 = mybir.dt.bfloat16
    nc = bacc.Bacc("TRN2", target_bir_lowering=False)

    gts = [
        nc.dram_tensor(f"gt{m}", [KP, KCH * JS], bf16, kind="ExternalInput")
        for m in range(4)
    ]
    # h_m in stationary layout: hs[m][p, 32k+c] = h_m[128k+p, c]
    hs = [
        nc.dram_tensor(f"h{m}", [KP, KCH * C], bf16, kind="ExternalInput")
        for m in range(4)
    ]
    # selector for the cross-strip reduction: sel[32m+c, c'] = delta(c, c')
    sel_d = nc.dram_tensor("sel", [KP, C], f32r, kind="ExternalInput")
    out_t = nc.dram_tensor("outT", [C, JS], f32, kind="ExternalOutput")

    with tile.TileContext(nc) as tc:
        with (
            tc.tile_pool(name="cpool", bufs=1) as cpool,
            tc.tile_pool(name="gpool", bufs=12) as gpool,
            tc.tile_pool(name="zpsum", bufs=2, space="PSUM") as zpsum,
            tc.tile_pool(name="opsum", bufs=2, space="PSUM") as opsum,
        ):
            # constants + replicated activations ride the HWDGE rings ahead
            # of the G stream (no gpsimd use -> no SWDGE drain in teardown)
            sel = cpool.tile([KP, C], f32r, tag="sel", name="sel")
            nc.sync.dma_start(sel[:], sel_d[:])
            h_sb = []
            for m in range(4):
                h = cpool.tile([KP, KCH * C], bf16, tag=f"h{m}", name=f"h{m}")
                (nc.sync if m % 2 == 0 else nc.scalar).dma_start(h[:], hs[m][:])
                h_sb.append(h)

            # zp[jh][32m:32m+32, j] accumulates branch m's partial
            # sum_t h_m[t, c] * G_m^T[t, j] over all 64 t-chunks; the four
            # branches run concurrently in the PE's four column strips.
            zp = [
                zpsum.tile([KP, 512], f32, tag=f"zp{jh}", name=f"zp{jh}")
                for jh in range(NJH)
            ]
            for kb in range(NBLK):
                gt_t = []
                for m in range(4):
                    gt = gpool.tile([KP, BLK * JS], bf16, tag="gt")
                    dma_eng = nc.sync if m % 2 == 0 else nc.scalar
                    dma_eng.dma_start(
                        gt[:], gts[m][:, BLK * JS * kb : BLK * JS * (kb + 1)]
                    )
                    gt_t.append(gt)
                for kk in range(BLK):
                    k = kb * BLK + kk
                    first = k == 0
                    last = k == KCH - 1
                    for m in range(4):
                        for jh in range(NJH):
                            nc.tensor.matmul(
                                zp[jh][32 * m : 32 * (m + 1), :],
                                h_sb[m][:, C * k : C * (k + 1)],
                                gt_t[m][
                                    :, JS * kk + 512 * jh : JS * kk + 512 * (jh + 1)
                                ],
                                start=first,
                                stop=last,
                                tile_position=(0, 32 * m),
                            )

            # cross-strip sum via selector matmul, then relu and store;
            # per-half so the first chain overlaps the other half's tail
            zcopy = cpool.tile([KP, NJH * 512], f32r, tag="zcopy")
            zsb = cpool.tile([C, JS], f32, tag="zsb")
            zo = [
                opsum.tile([C, 512], f32, tag=f"zo{jh}", name=f"zo{jh}")
                for jh in range(NJH)
            ]
            for jh in range(NJH):
                cp = zcopy[:, 512 * jh : 512 * (jh + 1)]
                nc.vector.tensor_copy(cp, zp[jh][:])
                nc.tensor.matmul(
                    zo[jh][:],
                    sel[:],
                    cp,
                    start=True,
                    stop=True,
                    tile_position=(0, 0),
                )
                nc.scalar.activation(
                    zsb[:, 512 * jh : 512 * (jh + 1)],
                    zo[jh][:],
                    mybir.ActivationFunctionType.Relu,
                )
                nc.sync.dma_start(
                    out_t[:, 512 * jh : 512 * (jh + 1)],
                    zsb[:, 512 * jh : 512 * (jh + 1)],
                )

    nc.compile()
    return nc


def _get_program():
    global _compiled
    if _compiled is None:
        _compiled = _build_program()
    return _compiled


def _prep_inputs(inputs):
    """Host-side sharding: returns per-core input maps."""
    from ml_dtypes import bfloat16 as bf16

    f32 = np.float32
    branches = [
        ("Gi2j", "xi", "W_i", "b_i"),
        ("Adj2j", "xj1", "W_j1", "b_j1"),
        ("coAdj2j", "xj1", "W_j2", "b_j2"),
        ("Gk2j", "xk", "W_k", "b_k"),
    ]
    shared = {"sel": np.tile(np.eye(C, dtype=f32), (KP // C, 1))}
    g_bf = []
    for m, (gn, xn, wn, bn) in enumerate(branches):
        x = np.asarray(inputs[xn], dtype=f32)
        w = np.asarray(inputs[wn], dtype=f32)
        b = np.asarray(inputs[bn], dtype=f32)
        h = (x @ w + b).astype(bf16)  # [N, C] replicated activation
        shared[f"h{m}"] = np.ascontiguousarray(
            h.reshape(KCH, KP, C).transpose(1, 0, 2).reshape(KP, KCH * C)
        )
        g_bf.append(np.asarray(inputs[gn], dtype=f32).astype(bf16))

    in_maps = []
    for s in range(N_CORES):
        im = dict(shared)
        for m in range(4):
            # gt[p, 1024k+j] = G[s*JS + j, 128k + p]
            shard_t = g_bf[m][s * JS : (s + 1) * JS, :].T  # [N(t), JS(j)]
            im[f"gt{m}"] = np.ascontiguousarray(
                shard_t.reshape(KCH, KP, JS).transpose(1, 0, 2).reshape(KP, KCH * JS)
            )
        in_maps.append(im)
    return in_maps


def _run(inputs, trace=False):
    from concourse.bass_utils import run_bass_kernel_spmd

    nc = _get_program()
    in_maps = _prep_inputs(inputs)
    try:
        res = run_bass_kernel_spmd(nc, in_maps, list(range(N_CORES)), trace=trace)
    except Exception:
        # transient device errors (e.g. NRT_EXEC_UNIT_UNRECOVERABLE) clear
        # on re-dispatch; retry once before giving up
        res = run_bass_kernel_spmd(nc, in_maps, list(range(N_CORES)), trace=trace)
    out = np.concatenate(
        [res.results[s]["outT"] for s in range(N_CORES)], axis=1
    ).T
    return np.ascontiguousarray(out, dtype=np.float32), res


def kernel(**inputs):
    out, _ = _run(inputs, trace=False)
    return out


# revision 7
# speedup vs baseline: 2.3217x; 1.2016x over previous
"""Trainium2 Bass kernel for nn_CXNGeneralLayer (GNN message passing).

z = relu(Gi2j @ (xi W_i + b_i) + Adj2j @ (xj1 W_j1 + b_j1)
         + coAdj2j @ (xj1 W_j2 + b_j2) + Gk2j @ (xk W_k + b_k))

Sharding (per the 1D row-parallel hint): output rows (n_j) are split
across 8 NeuronCores; each core streams its [1024, 8192] shard of all
four operator matrices, which dominate the traffic. Shards are
pre-transposed on the host (contraction dim on SBUF partitions) and
cast to bf16, halving HBM traffic to 64 MB/core; bf16 rounding
contributes ~2.3e-3 relative output error (measured offline), 8x
inside the 2e-2 gate. DMA tiles span 4 contraction chunks (1 MiB,
8 KiB contiguous per partition line) for near-line-rate streaming.

The PE array runs in 128x32 column-tiled mode: the four matrices'
matmuls execute CONCURRENTLY in the four 32-column strips
(tile_position=(0, 32m)), each strip accumulating its branch partial
h_m^T @ G_m^T into its own partition range of a shared PSUM bank.
This lifts array utilization 4x so the PE streams ~4x fewer wall-ns
than the naive layout and DMA becomes the sole bottleneck. The four
partials are then cross-partition summed by one selector matmul
(S[32m+c, c'] = delta(c, c')), relu'd, and stored.
"""

import sys

import numpy as np

if "/opt/trn_rl_repo" not in sys.path:
    sys.path.insert(0, "/opt/trn_rl_repo")

N = 8192  # n_i = n_j = n_k
C = 32  # c_in = c_out
N_CORES = 8
JS = N // N_CORES  # 1024 output rows per core
KP = 128  # contraction partition tile
KCH = N // KP  # 64 t-chunks
BLK = 4  # t-chunks per DMA tile (1 MiB bf16)
NBLK = KCH // BLK  # 16 block DMAs per matrix
NJH = 2  # j-halves of 512 (PSUM-bank moving-operand max)

_compiled = None


def _build_program():
    import concourse.mybir as mybir
    import concourse.tile as tile
    from concourse import bacc

    f32 = mybir.dt.float32
    f32r = mybir.dt.float32r
    bf16 = mybir.dt.bfloat16
    nc = bacc.Bacc("TRN2", target_bir_lowering=False)

    gts = [
        nc.dram_tensor(f"gt{m}", [KP, KCH * JS], bf16, kind="ExternalInput")
        for m in range(4)
    ]
    # h_m in stationary layout: hs[m][p, 32k+c] = h_m[128k+p, c]
    hs = [
        nc.dram_tensor(f"h{m}", [KP, KCH * C], bf16, kind="ExternalInput")
        for m in range(4)
    ]
    # selector for the cross-strip reduction: sel[32m+c, c'] = delta(c, c')
    sel_d = nc.dram_tensor("sel", [KP, C], f32r, kind="ExternalInput")
    out_t = nc.dram_tensor("outT", [C, JS], f32, kind="ExternalOutput")

    with tile.TileContext(nc) as tc:
        with (
            tc.tile_pool(name="cpool", bufs=1) as cpool,
            tc.tile_pool(name="gpool", bufs=12) as gpool,
            tc.tile_pool(name="tpool", bufs=16) as tpool,
            tc.tile_pool(name="zpsum", bufs=2, space="PSUM") as zpsum,
            tc.tile_pool(name="opsum", bufs=2, space="PSUM") as opsum,
        ):
            # ring pairing: sync streams m0/m1, scalar streams m2/m3 so each
            # HWDGE ring reads long sequential runs from just two tensors
            ring = [nc.sync, nc.sync, nc.scalar, nc.scalar]

            # first G block leads the rings so the HBM stream starts
            # immediately; sel + h staging rides behind it (the PE has
            # ~100 us of slack, so the later h arrival costs nothing)
            gt_t0 = []
            for m in range(4):
                gt = gpool.tile([KP, BLK * JS], bf16, tag="gt")
                ring[m].dma_start(gt[:], gts[m][:, : BLK * JS])
                gt_t0.append(gt)

            sel = cpool.tile([KP, C], f32r, tag="sel", name="sel")
            nc.sync.dma_start(sel[:], sel_d[:])
            h_sb = []
            for m in range(4):
                h = cpool.tile([KP, KCH * C], bf16, tag=f"h{m}", name=f"h{m}")
                ring[m].dma_start(h[:], hs[m][:])
                h_sb.append(h)

            # zp[jh][32m:32m+32, j] accumulates branch m's partial
            # sum_t h_m[t, c] * G_m^T[t, j] over all 64 t-chunks; the four
            # branches run concurrently in the PE's four column strips.
            zp = [
                zpsum.tile([KP, 512], f32, tag=f"zp{jh}", name=f"zp{jh}")
                for jh in range(NJH)
            ]

            def chunk_mms(k, gt_slices):
                first = k == 0
                last = k == KCH - 1
                for m in range(4):
                    for jh in range(NJH):
                        nc.tensor.matmul(
                            zp[jh][32 * m : 32 * (m + 1), :],
                            h_sb[m][:, C * k : C * (k + 1)],
                            gt_slices[m][:, 512 * jh : 512 * (jh + 1)],
                            start=first,
                            stop=last,
                            tile_position=(0, 32 * m),
                        )

            for kb in range(NBLK - 1):
                gt_t = gt_t0
                if gt_t is None:
                    gt_t = []
                    for m in range(4):
                        gt = gpool.tile([KP, BLK * JS], bf16, tag="gt")
                        ring[m].dma_start(
                            gt[:], gts[m][:, BLK * JS * kb : BLK * JS * (kb + 1)]
                        )
                        gt_t.append(gt)
                gt_t0 = None
                for kk in range(BLK):
                    k = kb * BLK + kk
                    chunk_mms(
                        k, [gt[:, JS * kk : JS * (kk + 1)] for gt in gt_t]
                    )

            # last block streams per-chunk (256 KB DMAs) so the PE trails
            # the end of the HBM stream by ~1 chunk instead of 4
            for kk in range(BLK):
                k = (NBLK - 1) * BLK + kk
                gt_c = []
                for m in range(4):
                    gt = tpool.tile([KP, JS], bf16, tag="gtail")
                    ring[m].dma_start(gt[:], gts[m][:, JS * k : JS * (k + 1)])
                    gt_c.append(gt)
                chunk_mms(k, [gt[:] for gt in gt_c])

            # cross-strip sum via selector matmul, then relu and store;
            # per-half so the first chain overlaps the other half's tail
            zcopy = cpool.tile([KP, NJH * 512], f32r, tag="zcopy")
            zsb = cpool.tile([C, JS], f32, tag="zsb")
            zo = [
                opsum.tile([C, 512], f32, tag=f"zo{jh}", name=f"zo{jh}")
                for jh in range(NJH)
            ]
            for jh in range(NJH):
                cp = zcopy[:, 512 * jh : 512 * (jh + 1)]
                nc.vector.tensor_copy(cp, zp[jh][:])
                nc.tensor.matmul(
                    zo[jh][:],
                    sel[:],
                    cp,
                    start=True,
                    stop=True,
                    tile_position=(0, 0),
                )
                nc.scalar.activation(
                    zsb[:, 512 * jh : 512 * (jh + 1)],
                    zo[jh][:],
                    mybir.ActivationFunctionType.Relu,
                )
                nc.sync.dma_start(
                    out_t[:, 512 * jh : 512 * (jh + 1)],
                    zsb[:, 512 * jh : 512 * (jh + 1)],
                )

    nc.compile()
    return nc


def _get_program():
    global _compiled
    if _compiled is None:
        _compiled = _build_program()
    return _compiled


def _prep_inputs(inputs):
    """Host-side sharding: returns per-core input maps."""
    from ml_dtypes import bfloat16 as bf16

    f32 = np.float32
    branches = [
        ("Gi2j", "xi", "W_i", "b_i"),
        ("Adj2j", "xj1", "W_j1", "b_j1"),
        ("coAdj2j", "xj1", "W_j2", "b_j2"),
        ("Gk2j", "xk", "W_k", "b_k"),
    ]
    shared = {"sel": np.tile(np.eye(C, dtype=f32), (KP // C, 1))}
    g_bf = []
    for m, (gn, xn, wn, bn) in enumerate(branches):
        x = np.asarray(inputs[xn], dtype=f32)
        w = np.asarray(inputs[wn], dtype=f32)
        b = np.asarray(inputs[bn], dtype=f32)
        h = (x @ w + b).astype(bf16)  # [N, C] replicated activation
        shared[f"h{m}"] = np.ascontiguousarray(
            h.reshape(KCH, KP, C).transpose(1, 0, 2).reshape(KP, KCH * C)
        )
        g_bf.append(np.asarray(inputs[gn], dtype=f32).astype(bf16))

    in_maps = []
    for s in range(N_CORES):
        im = dict(shared)
        for m in range(4):
            # gt[p, 1024k+j] = G[s*JS + j, 128k + p]
            shard_t = g_bf[m][s * JS : (s + 1) * JS, :].T  # [N(t), JS(j)]
            im[f"gt{m}"] = np.ascontiguousarray(
                shard_t.reshape(KCH, KP, JS).transpose(1, 0, 2).reshape(KP, KCH * JS)
            )
        in_maps.append(im)
    return in_maps


def _run(inputs, trace=False):
    from concourse.bass_utils import run_bass_kernel_spmd

    nc = _get_program()
    in_maps = _prep_inputs(inputs)
    try:
        res = run_bass_kernel_spmd(nc, in_maps, list(range(N_CORES)), trace=trace)
    except Exception:
        # transient device errors (e.g. NRT_EXEC_UNIT_UNRECOVERABLE) clear
        # on re-dispatch; retry once before giving up
        res = run_bass_kernel_spmd(nc, in_maps, list(range(N_CORES)), trace=trace)
    out = np.concatenate(
        [res.results[s]["outT"] for s in range(N_CORES)], axis=1
    ).T
    return np.ascontiguousarray(out, dtype=np.float32), res


def kernel(**inputs):
    out, _ = _run(inputs, trace=False)
    return out
